# revision 1
# baseline (speedup 1.0000x reference)
"""Trainium2 Bass kernel for nn_DecoderSplatting.

Per-pixel gaussian-splat decoding over (8 views, 480x640): ray directions
from inverse intrinsics, depth from sigmoid(disp), softplus rgb/scales,
sigmoid opacity, and world-frame quaternion = extrinsic-quat (x) cam-quat
with a Shepperd-branch sign fix.  One view per NeuronCore (8 cores, SPMD);
each core streams its view in 4 row-chunks of [120 partitions x 640].

Math notes (validated against the jax reference, absmax ~2e-6):
- inv(K) here is diagonal-ish (zero skew): dirs ∝ [e0, e1, c2] with
  e0 = sigmoid(ox) + (x - .5 + A02/A00), e1 = r11*sigmoid(oy) + gy,
  c2 = A22/A00; normalization absorbs the A00 scale.
- means = t + (R @ e) * exp(-0.5*(ln(|e|^2) + 2*ln(k1*sig(disp)+k2)))
- world quat: wq = M_E @ q_cam (4x4 constant per view), normalized, with
  sign flipped iff min(wq)^2 == max(wq_i^2)  (i.e. the largest-|.|
  component is negative) — matches scipy/Shepperd pivot sign.
- All ACT transcendentals use only {Exp, Ln, Square, Copy} => a single
  activation table set (natural_log_exp_and_others), no table thrash.
"""

import sys

import numpy as np

try:
    import concourse.bass as bass
except ImportError:  # pragma: no cover
    sys.path.insert(0, "/opt/trn_rl_repo")
    import concourse.bass as bass

import concourse.bacc as bacc

import concourse.mybir as mybir
from concourse.tile import TileContext

F32 = mybir.dt.float32
Alu = mybir.AluOpType
Act = mybir.ActivationFunctionType

NEAR, FAR = 0.05, 20.0
K1 = float(1.0 / NEAR - 1.0 / FAR)
K2 = float(1.0 / FAR)

V = 8
C = 14
H = 480
W = 640
P = 120          # partitions per chunk
NCHUNK = H // P  # 4
NCST = 32

_CACHE = {}


class _CoveringSetBacc(bacc.Bacc):
    """Bacc whose act-table-load pass collapses to one covering table set.

    The stock pass assigns each activation the *first* table set containing
    its function (Exp -> exp_and_others, Ln -> natural_log), which ping-pongs
    a ~2.7us table load before nearly every activation.  All functions used
    here live in natural_log_exp_and_others, so rewrite every load to that
    covering set and drop the duplicates (the loads carry no sync info).
    """

    def insert_act_table_loads(self):
        super().insert_act_table_loads()
        from concourse.hw_specs import get_activation_tables

        tables = list(get_activation_tables(self.m.arch).items())
        used = set()
        for b in self.main_func.blocks:
            for i in b.instructions:
                if isinstance(i, mybir.InstActivation):
                    used.add(i.func)
        cover = None
        for idx, (_, funcs) in enumerate(tables):
            if used <= funcs:
                cover = idx
                break
        if cover is None:
            return
        for b in self.main_func.blocks:
            seen = False
            keep = []
            for i in b.instructions:
                if isinstance(i, mybir.InstLoadActFuncSet):
                    if seen:
                        continue
                    i.act_func_set_id = cover
                    seen = True
                keep.append(i)
            b.instructions[:] = keep


def _build_nc():
    nc = _CoveringSetBacc()
    raw = nc.dram_tensor("raw", [C, H, W], F32, kind="ExternalInput")
    gx = nc.dram_tensor("gx", [P, W], F32, kind="ExternalInput")
    gyt = nc.dram_tensor("gyt", [P, NCHUNK], F32, kind="ExternalInput")
    cst = nc.dram_tensor("cst", [P, NCST], F32, kind="ExternalInput")
    out = nc.dram_tensor("out", [H, W, 15], F32, kind="ExternalOutput")

    va = nc.vector
    ae = nc.scalar
    ge = nc.gpsimd

    with TileContext(nc) as tc:
        with (
            tc.tile_pool(name="inp", bufs=2) as in_pool,
            tc.tile_pool(name="outp", bufs=2) as out_pool,
            tc.tile_pool(name="scr", bufs=6) as scr_pool,
            tc.tile_pool(name="scr2", bufs=4) as scr2_pool,
            tc.tile_pool(name="named", bufs=1) as named_pool,
            tc.tile_pool(name="consts", bufs=1) as cst_pool,
        ):
            gx_t = cst_pool.tile([P, W], F32, tag="gx", name="gx_t")
            nc.sync.dma_start(out=gx_t[:], in_=gx[:])
            gyt_t = cst_pool.tile([P, NCHUNK], F32, tag="gyt", name="gyt_t")
            nc.sync.dma_start(out=gyt_t[:], in_=gyt[:])
            cst_t = cst_pool.tile([P, NCST], F32, tag="cst_t", name="cst_t")
            nc.sync.dma_start(out=cst_t[:], in_=cst[:])

            def CST(i):
                return cst_t[:, i:i + 1]

            def vtile():
                return scr_pool.tile([P, W], F32, tag="vscr", name="vscr")

            def v2tile():
                return scr2_pool.tile([P, 2 * W], F32, tag="vscr2", name="vscr2")

            def ntile(tg, fw=1, nb=1):
                return named_pool.tile([P, fw * W], F32, tag=tg, name=tg,
                                       bufs=nb)

            for k in range(NCHUNK):
                rows = raw[:, k * P:(k + 1) * P, :]
                ITQ = in_pool.tile([P, 4 * W], F32, tag="ITQ", name="ITQ")
                nc.sync.dma_start(
                    out=ITQ[:].rearrange("p (c w) -> p c w", c=4),
                    in_=rows[8:12].rearrange("c p w -> p c w"),
                )
                ITA = in_pool.tile([P, 8 * W], F32, tag="ITA", name="ITA")
                nc.sync.dma_start(
                    out=ITA[:].rearrange("p (c w) -> p c w", c=8),
                    in_=rows[0:8].rearrange("c p w -> p c w"),
                )
                ITX = in_pool.tile([P, 2 * W], F32, tag="ITX", name="ITX")
                nc.sync.dma_start(
                    out=ITX[:].rearrange("p (c w) -> p c w", c=2),
                    in_=rows[12:14].rearrange("c p w -> p c w"),
                )
                OT = out_pool.tile([P, 15 * W], F32, tag="OT", name="OT")
                itva = ITA[:].rearrange("p (c w) -> p c w", c=8)
                otv = OT[:].rearrange("p (w q) -> p w q", q=15)

                def ch(c):
                    return itva[:, c, :]

                def st(q):
                    return otv[:, :, q]

                def stm(q0, n):
                    # n adjacent out channels, channel-major enumeration
                    return otv[:, :, q0:q0 + n].rearrange("p w q -> p q w")

                # --- quat matvec: wq columns of one wide tile
                wqt = ntile("wqt", 4)
                wq = [wqt[:, i * W:(i + 1) * W] for i in range(4)]
                qsrc = [ITQ[:, i * W:(i + 1) * W] for i in range(4)]
                for i in range(4):
                    a = vtile()
                    ge.tensor_scalar(a, qsrc[0], CST(15 + 4 * i), None,
                                     Alu.mult)
                    va.scalar_tensor_tensor(a, qsrc[1], CST(16 + 4 * i), a,
                                            Alu.mult, Alu.add)
                    va.scalar_tensor_tensor(a, qsrc[2], CST(17 + 4 * i), a,
                                            Alu.mult, Alu.add)
                    va.scalar_tensor_tensor(wq[i], qsrc[3], CST(18 + 4 * i), a,
                                            Alu.mult, Alu.add)
                mn12 = v2tile()
                va.tensor_tensor(mn12, wqt[:, :2 * W], wqt[:, 2 * W:], Alu.min)
                mn = ntile("mn")
                va.tensor_tensor(mn, mn12[:, :W], mn12[:, W:], Alu.min)

                # --- rgb softplus (pair + single) -> out ch4..6
                a2 = v2tile()
                ae.activation(a2, ITA[:, 0:2 * W], Act.Exp)
                ae.activation(stm(4, 2), a2, Act.Ln, bias=1.0)
                a1 = vtile()
                ae.activation(a1, ch(2), Act.Exp)
                ae.activation(st(6), a1, Act.Ln, bias=1.0)

                # --- xy sigmoid, both channels in one chain
                sxy = ntile("sxy", 2)
                s_a = v2tile()
                ae.activation(s_a, ITX[:], Act.Exp, scale=-1.0)
                s_b = v2tile()
                ae.activation(s_b, s_a, Act.Ln, bias=1.0)
                ae.activation(sxy, s_b, Act.Exp, scale=-1.0)
                e01 = ntile("e01", 2)
                va.tensor_tensor(e01[:, :W], sxy[:, :W], gx_t[:], Alu.add)
                va.tensor_scalar(e01[:, W:], sxy[:, W:], CST(1),
                                 gyt_t[:, k:k + 1], Alu.mult, Alu.add)

                # --- scales softplus * mult (pair + single) -> out ch8..10
                b2 = v2tile()
                ae.activation(b2, ITA[:, 5 * W:7 * W], Act.Exp)
                ae.activation(b2, b2, Act.Ln, bias=1.0)
                ge.tensor_scalar(stm(8, 2), b2, CST(14), None, Alu.mult)
                b1 = vtile()
                ae.activation(b1, ch(7), Act.Exp)
                ae.activation(b1, b1, Act.Ln, bias=1.0)
                ge.tensor_scalar(st(10), b1, CST(14), None, Alu.mult)

                # --- disp+opacity (adjacent ch3,ch4): E = exp(-x), L = ln(1+E)
                E34 = v2tile()
                ae.activation(E34, ITA[:, 3 * W:5 * W], Act.Exp, scale=-1.0)
                L34 = v2tile()
                ae.activation(L34, E34, Act.Ln, bias=1.0)
                ae.activation(st(7), L34[:, W:], Act.Exp, scale=-1.0)  # opac
                la = vtile()
                ae.activation(la, E34[:, :W], Act.Ln, scale=K2, bias=CST(31))

                # --- quat norm + sign fix (squares into dead ITQ)
                smn = vtile()
                ae.activation(smn, mn, Act.Square)
                ge.tensor_tensor(ITQ[:], wqt, wqt, Alu.mult)      # sq all 4
                sq = [ITQ[:, i * W:(i + 1) * W] for i in range(4)]
                n12 = v2tile()
                va.tensor_tensor(n12, ITQ[:, :2 * W], ITQ[:, 2 * W:], Alu.add)
                m12 = v2tile()
                va.tensor_tensor(m12, ITQ[:, :2 * W], ITQ[:, 2 * W:], Alu.max)
                va.tensor_tensor(m12[:, :W], m12[:, :W], m12[:, W:], Alu.max)
                va.tensor_tensor(n12[:, :W], n12[:, :W], n12[:, W:], Alu.add)
                va.tensor_tensor(sq[1], smn, m12[:, :W], Alu.is_equal)
                ge.tensor_scalar(sq[1], sq[1], -2.0, 1.0, Alu.mult, Alu.add)
                ae.activation(sq[3], n12[:, :W], Act.Ln)
                ae.activation(sq[2], sq[3], Act.Exp, scale=-0.5)  # 1/|wq|
                va.tensor_tensor(sq[2], sq[2], sq[1], Alu.mult)   # isv
                for j in range(4):
                    ge.tensor_tensor(st(11 + j), wq[j], sq[2], Alu.mult)

                # --- ray norm + depth -> sfac
                t01 = v2tile()
                ae.activation(t01, e01, Act.Square)
                va.tensor_tensor(t01[:, :W], t01[:, :W], t01[:, W:], Alu.add)
                ae.activation(t01[:, W:], t01[:, :W], Act.Ln, bias=CST(0))
                lb = L34[:, :W]
                va.tensor_tensor(la, la, lb, Alu.subtract)        # ld
                va.scalar_tensor_tensor(la, la, 2.0, t01[:, W:],
                                        Alu.mult, Alu.add)
                sfac = ntile("sfac")
                ae.activation(sfac, la, Act.Exp, scale=-0.5)

                # --- means: m_i = f0*Ri0 + f1*Ri1 + (sfac*Ri2c2 + t_i)
                for i in range(3):
                    wa = vtile()
                    ge.tensor_scalar(wa, e01[:, W:], CST(5 + i), CST(8 + i),
                                     Alu.mult, Alu.add)
                    va.scalar_tensor_tensor(wa, e01[:, :W], CST(2 + i), wa,
                                            Alu.mult, Alu.add)
                    va.tensor_tensor(wa, wa, sfac, Alu.mult)
                    va.tensor_scalar(st(i), wa, CST(11 + i), None, Alu.add)
                ae.activation(st(3), gx_t[:], Act.Copy, bias=1.0, scale=0.0)

                nc.scalar.dma_start(
                    out=out[k * P:(k + 1) * P].rearrange("p w q -> p (w q)"),
                    in_=OT[:],
                )
    nc.finalize()
    return nc


def _mat_to_quat_wxyz(m):
    m = np.asarray(m, np.float64)
    m00, m01, m02 = m[0, 0], m[0, 1], m[0, 2]
    m10, m11, m12 = m[1, 0], m[1, 1], m[1, 2]
    m20, m21, m22 = m[2, 0], m[2, 1], m[2, 2]
    tr = m00 + m11 + m22
    qs = [
        np.array([m21 - m12, 1 + m00 - m11 - m22, m01 + m10, m02 + m20]),
        np.array([m02 - m20, m01 + m10, 1 + m11 - m00 - m22, m12 + m21]),
        np.array([m10 - m01, m02 + m20, m12 + m21, 1 + m22 - m00 - m11]),
        np.array([1 + tr, m21 - m12, m02 - m20, m10 - m01]),
    ]
    q = qs[int(np.argmax([m00, m11, m22, tr]))]
    return q / np.linalg.norm(q)


def _per_view_inputs(raw_v, E, K):
    """Host-side per-view constants -> the in_map for one core."""
    A = np.linalg.inv(K.astype(np.float32))
    a00 = float(A[0, 0])
    assert a00 > 0
    assert abs(A[0, 1]) < 1e-6 * a00 and abs(A[1, 0]) < 1e-6 * a00
    assert abs(A[2, 0]) < 1e-9 and abs(A[2, 1]) < 1e-9
    assert np.allclose(E[3], [0, 0, 0, 1], atol=1e-6)
    R = E[:3, :3].astype(np.float64)
    t = E[:3, 3].astype(np.float64)
    c2 = float(A[2, 2]) / a00
    r11 = float(A[1, 1]) / a00
    mult = float(np.linalg.inv(K[:2, :2].astype(np.float32)).sum())

    ew, ex, ey, ez = _mat_to_quat_wxyz(R)
    M = np.array([
        [-ex, -ey, -ez, ew],
        [ew, -ez, ey, ex],
        [ez, ew, -ex, ey],
        [-ey, ex, ew, ez],
    ], np.float64)

    cstv = np.zeros(NCST, np.float64)
    cstv[0] = c2 * c2
    cstv[1] = r11
    for i in range(3):
        cstv[2 + i] = R[i, 0]
        cstv[5 + i] = R[i, 1]
        cstv[8 + i] = R[i, 2] * c2
        cstv[11 + i] = t[i]
    cstv[14] = mult
    cstv[15:31] = M.reshape(-1)
    cstv[31] = K1 + K2   # bias for ln(k2*E + (k1+k2))
    cst = np.broadcast_to(cstv.astype(np.float32), (P, NCST)).copy()

    xs = np.arange(W, dtype=np.float32)
    gxrow = (xs - np.float32(0.5)) + np.float32(float(A[0, 2]) / a00)
    gx = np.broadcast_to(gxrow, (P, W)).copy()

    ys = np.arange(H, dtype=np.float32)
    gycol = np.float32(r11) * (ys - np.float32(0.5)) + \
        np.float32(float(A[1, 2]) / a00)
    gyt = gycol.reshape(NCHUNK, P).T.copy()  # [P, NCHUNK]

    return {
        "raw": np.ascontiguousarray(raw_v, np.float32),
        "gx": gx,
        "gyt": np.ascontiguousarray(gyt, np.float32),
        "cst": cst,
    }


def kernel(raw_gaussians, extrinsics, intrinsics, _trace=False, _trace_kwargs=None):
    raw_gaussians = np.asarray(raw_gaussians, np.float32)
    extrinsics = np.asarray(extrinsics, np.float32)
    intrinsics = np.asarray(intrinsics, np.float32)
    b, v, c, h, w = raw_gaussians.shape
    assert (b, v, c, h, w) == (1, V, C, H, W), raw_gaussians.shape

    if "nc" not in _CACHE:
        _CACHE["nc"] = _build_nc()
    nc = _CACHE["nc"]

    in_maps = [
        _per_view_inputs(raw_gaussians[0, vi], extrinsics[0, vi],
                         intrinsics[0, vi])
        for vi in range(V)
    ]

    from concourse.bass_utils import run_bass_kernel_spmd

    kwargs = {}
    if _trace:
        kwargs.update(trace=True, **(_trace_kwargs or {}))
    res = run_bass_kernel_spmd(nc, in_maps, core_ids=list(range(V)), **kwargs)
    out = np.stack([res.results[i]["out"] for i in range(V)], axis=0)
    if _trace:
        _CACHE["last_results"] = res
    return out

